# revision 10
# baseline (speedup 1.0000x reference)
"""Trainium2 Bass kernel for MildenhallNeRF hash-grid encode + MLP.

Strategy (8 NeuronCores, pure data parallel over B):
  - Each core gets B/8 = 262144 points; hash tables + MLP weights replicated.
  - Phase 0: cast tables to bf16 (embB) and build overlapping z-pair tables
    (embB2) for the two dense levels via SBUF-side DVE interleave (the DRAM
    strided interleave costs ~14ms in SWDGE element descriptors).
  - Phase 1 (level-major): per level broadcast the table to all 128 SBUF
    partitions, then stream point tiles: DVE computes voxel coords / hash
    indices / trilinear weights from an SBUF-resident scaled-coordinate
    cache (filled once at level 0), GPSIMD ap_gather fetches corner
    features (the Pool-engine RD_CMD rate of ~33 cycles/index is the
    hardware floor and dominates the kernel), DVE masks the 16x core
    replication, applies trilinear weights and reduces. Gather outputs go
    to two statically allocated SBUF buffers (manual double buffering).
    Per-level features land in a DRAM scratch laid out feature-major.
  - Phase 2: tiny MLP on TensorE (bf16, N=512 tiles), ACT for bias+relu/
    sigmoid. Output is produced transposed [4, B/8]; host reassembles.
"""
import sys
import numpy as np

for _p in ("/opt/trn_rl_repo", "/root/.axon_site/_ro/trn_rl_repo"):
    if _p not in sys.path:
        sys.path.append(_p)

import concourse.bass as bass
import concourse.tile as tile
from concourse import mybir, bacc

L = 16
TBL = 16384
B_GROWTH = np.exp((np.log(512.0) - np.log(16.0)) / (L - 1))
NS = [int(16 * B_GROWTH**i) for i in range(L)]
P1, P2 = 2654435761, 805459861
PM1, PM2 = P1 & 16383, P2 & 16383
BB_MIN, BB_SIZE = -5.0, 10.0
CLAMP_HI = float(np.float32(1.0) - np.float32(1e-6))

B_FULL = 2097152
NCORES = 8
B_NC = B_FULL // NCORES            # 262144
P = 128
R = 64                              # points per partition per tile
TPTS = P * R                        # 8192 points per tile
NIDX = 16 * 8 * R                   # 8192 indices per Q7 core per gather
NM = 512                            # MLP tile width
FROWS = 35
FCOLS = FROWS * R                   # 2240 bf16 elements per partition per tile

f32 = mybir.dt.float32
bf16 = mybir.dt.bfloat16
i32 = mybir.dt.int32
i16 = mybir.dt.int16

PAIR_LEVELS = [l for l in range(L) if (NS[l] + 1) ** 3 <= TBL]  # [0, 1]


def build(b_nc=B_NC, skip_gather=False, skip_ph2=False, skip_prep=False,
          n_levels=L):
    nt = b_nc // TPTS
    nmt = 0 if skip_ph2 else b_nc // NM
    nc = bacc.Bacc("TRN2", target_bir_lowering=False, debug=False)

    x_d = nc.dram_tensor("x", [b_nc, 6], f32, kind="ExternalInput")
    emb_d = nc.dram_tensor("embed", [L, TBL, 2], f32, kind="ExternalInput")
    w_d = {}
    for name, shp in [("dW0", [32, 64]), ("db0", [64]), ("dW1", [64, 16]),
                      ("db1", [16]), ("cW0", [19, 64]), ("cb0", [64]),
                      ("cW1", [64, 64]), ("cb1", [64]), ("cW2", [64, 3]),
                      ("cb2", [3])]:
        w_d[name] = nc.dram_tensor(name, shp, f32, kind="ExternalInput")
    out_d = nc.dram_tensor("out", [4, b_nc], f32, kind="ExternalOutput")
    ptsF = nc.dram_tensor("ptsF", [nt, P, FCOLS], bf16, kind="Internal")
    embB = nc.dram_tensor("embB", [L, TBL * 2], bf16, kind="Internal")
    # overlapping z-pair tables for the dense levels: T2[e] = (T[e], T[e+1])
    NPAD = [256 * -(-(((NS[l] + 1) ** 3) * 2) // 256) for l in PAIR_LEVELS]
    embB2 = nc.dram_tensor("embB2", [len(PAIR_LEVELS), 2 * max(NPAD)], bf16,
                           kind="Internal")
    # level-0 (y,z)-quad table: Q4[e] = (T[e], T[e+1], T[e+n+1], T[e+n+2])
    NPADQ = NPAD[0]
    QSH = 2 * (NS[0] + 1)              # elem shift for the v1+1 corners
    embB4 = nc.dram_tensor("embB4", [4 * NPADQ], bf16, kind="Internal")

    # constants
    maskc_np = np.zeros((P, 16, 2), np.float32)
    for p in range(P):
        maskc_np[p, p % 16, :] = 1.0
    maskc_d = nc.inline_tensor(maskc_np.reshape(P, 32), name="maskc")
    maskc4_np = np.zeros((P, 16, 4), np.float32)
    for p in range(P):
        maskc4_np[p, p % 16, :] = 1.0
    maskc4_d = nc.inline_tensor(maskc4_np.reshape(P, 64), name="maskc4")
    maskc8_np = np.zeros((P, 16, 8), np.float32)
    for p in range(P):
        maskc8_np[p, p % 16, :] = 1.0
    maskc8_d = nc.inline_tensor(maskc8_np.reshape(P, 128), name="maskc8")

    pm_all = np.zeros((L, P, 6), np.float32)
    for l in range(L):
        n = NS[l]
        if (n + 1) ** 3 <= TBL:
            pm_all[l, :, :] = np.array([(n + 1) ** 2] * 2 + [n + 1] * 2 + [1, 1], np.float32)
        else:
            pm_all[l, :, :] = np.array([1, 1, PM1, PM1, PM2, PM2], np.float32)
    pm_d = nc.inline_tensor(pm_all, name="pmconst")

    xv = x_d.rearrange("(t p r) c -> t p r c", p=P, r=R)

    with tile.TileContext(nc) as tc:
        # ---------------- Phase 0: table prep ----------------
        with tc.tile_pool(name="pre", bufs=2) as prepool:
            # f32 -> bf16 cast of all tables (SWDGE cast via SBUF)
            for lc in range(L):
                stg = prepool.tile([P, TBL * 2 // P], bf16, name="stg")
                nc.gpsimd.dma_start(
                    out=stg[:],
                    in_=emb_d[lc].rearrange("a b -> (a b)").rearrange("(p q) -> p q", p=P))
                nc.sync.dma_start(
                    out=embB[lc].rearrange("(p q) -> p q", p=P), in_=stg[:])
            # z-pair tables, interleaved on DVE in SBUF (chunked across
            # partitions; stgB is the one-entry-shifted copy so pair k of
            # chunk p is (T[e], T[e+1]) with e = p*C/2 + k)
            for li, lc in enumerate(PAIR_LEVELS):
                C = NPAD[li] // P          # elems per partition, even
                stgA = prepool.tile([P, C], bf16, name=f"stgA{li}")
                nc.sync.dma_start(
                    out=stgA[:],
                    in_=embB[lc, 0:NPAD[li]].rearrange("(p q) -> p q", p=P))
                stgB = prepool.tile([P, C], bf16, name=f"stgB{li}")
                nc.sync.dma_start(
                    out=stgB[:],
                    in_=embB[lc, 2:NPAD[li] + 2].rearrange("(p q) -> p q", p=P))
                pairT = prepool.tile([P, C // 2, 2, 2], bf16, name=f"pair{li}")
                nc.vector.tensor_copy(
                    out=pairT[:, :, 0, :],
                    in_=stgA[:].rearrange("p (k e) -> p k e", e=2))
                nc.vector.tensor_copy(
                    out=pairT[:, :, 1, :],
                    in_=stgB[:].rearrange("p (k e) -> p k e", e=2))
                nc.sync.dma_start(
                    out=embB2[li, 0:2 * NPAD[li]].rearrange("(p q) -> p q", p=P),
                    in_=pairT[:].rearrange("p k a e -> p (k a e)"))
            # level-0 quad table: 4 shifted chunked loads + DVE interleave
            CQ = NPADQ // P
            quadT = prepool.tile([P, CQ // 2, 4, 2], bf16, name="quadT")
            for k, sh in enumerate([0, 2, QSH, QSH + 2]):
                stgQ = prepool.tile([P, CQ], bf16, name=f"stgQ{k}")
                nc.sync.dma_start(
                    out=stgQ[:],
                    in_=embB[0, sh:NPADQ + sh].rearrange("(p q) -> p q", p=P))
                nc.vector.tensor_copy(
                    out=quadT[:, :, k, :],
                    in_=stgQ[:].rearrange("p (k e) -> p k e", e=2))
            nc.sync.dma_start(
                out=embB4[0:4 * NPADQ].rearrange("(p q) -> p q", p=P),
                in_=quadT[:].rearrange("p k a e -> p (k a e)"))

        # ---------------- Phase 1: hash-grid encode ----------------
        with tc.tile_pool(name="ph1c", bufs=1) as cpool, \
             tc.tile_pool(name="ph1m", bufs=2) as mpool, \
             tc.tile_pool(name="ph1w", bufs=2) as pool:
            maskt = cpool.tile([P, 32], bf16, name="maskt")
            nc.gpsimd.dma_start(out=maskt[:], in_=maskc_d[:, :])
            maskt4 = cpool.tile([P, 64], bf16, name="maskt4")
            nc.gpsimd.dma_start(out=maskt4[:], in_=maskc4_d[:, :])
            maskt8 = cpool.tile([P, 128], bf16, name="maskt8")
            nc.gpsimd.dma_start(out=maskt8[:], in_=maskc8_d[:, :])
            # static double-buffered gather outputs
            gbuf = [cpool.tile([P, NIDX * 2], bf16, name=f"gb{j}")
                    for j in range(2)]

            def consume(gc, w8c, tc_, lc_, dense_):
                res = pool.tile([P, 2, R], f32, name="res")
                if dense_ == "quad":
                    # gather out: [p, (c=2, r) (i=16) (u=4, f=2)]
                    gv8 = gc[:].rearrange("p (rc v) -> p rc v", v=128)
                    nc.vector.tensor_tensor(
                        out=gv8, in0=gv8,
                        in1=maskt8[:].rearrange("p v -> p () v").to_broadcast([P, R * 2, 128]),
                        op=mybir.AluOpType.mult)
                    G8 = mpool.tile([P, R * 2, 8], f32, name="G")
                    nc.vector.tensor_reduce(
                        out=G8[:],
                        in_=gc[:].rearrange("p (rc i uf) -> p rc uf i", i=16, uf=8),
                        axis=mybir.AxisListType.X, op=mybir.AluOpType.add)
                    # weight every (c,u) corner and reduce over u then c
                    nc.vector.tensor_tensor(
                        out=G8[:].rearrange("p s (u f) -> p s u f", u=4),
                        in0=G8[:].rearrange("p s (u f) -> p s u f", u=4),
                        in1=w8c[:].rearrange("p c r u -> p (c r) u ()")
                            .to_broadcast([P, R * 2, 4, 2]),
                        op=mybir.AluOpType.mult)
                    S = mpool.tile([P, R * 2, 2], f32, name="S")
                    nc.vector.tensor_reduce(
                        out=S[:],
                        in_=G8[:].rearrange("p s (u f) -> p s f u", u=4),
                        axis=mybir.AxisListType.X, op=mybir.AluOpType.add)
                    nc.vector.tensor_reduce(
                        out=res[:].rearrange("p f r -> p r f"),
                        in_=S[:].rearrange("p (c r) f -> p r f c", c=2),
                        axis=mybir.AxisListType.X, op=mybir.AluOpType.add)
                elif dense_:
                    gv4 = gc[:].rearrange("p (rc v) -> p rc v", v=64)
                    nc.vector.tensor_tensor(
                        out=gv4, in0=gv4,
                        in1=maskt4[:].rearrange("p v -> p () v").to_broadcast([P, R * 4, 64]),
                        op=mybir.AluOpType.mult)
                    G4 = mpool.tile([P, R * 4, 4], f32, name="G")
                    nc.vector.tensor_reduce(
                        out=G4[:],
                        in_=gc[:].rearrange("p (rc i pf) -> p rc pf i", i=16, pf=4),
                        axis=mybir.AxisListType.X, op=mybir.AluOpType.add)
                    G4c = G4[:].rearrange("p (c r) (z f) -> p c r z f", c=4, z=2)
                    w8v = w8c[:].rearrange("p (a b) r -> p a b r", b=2)
                    nc.vector.tensor_tensor(
                        out=G4c[:, :, :, 0, :], in0=G4c[:, :, :, 0, :],
                        in1=w8v[:, :, 0, :].rearrange("p c r -> p c r ()")
                            .to_broadcast([P, 4, R, 2]),
                        op=mybir.AluOpType.mult)
                    nc.vector.tensor_tensor(
                        out=G4c[:, :, :, 1, :], in0=G4c[:, :, :, 1, :],
                        in1=w8v[:, :, 1, :].rearrange("p c r -> p c r ()")
                            .to_broadcast([P, 4, R, 2]),
                        op=mybir.AluOpType.mult)
                    nc.vector.tensor_tensor(
                        out=G4c[:, :, :, 0, :], in0=G4c[:, :, :, 0, :],
                        in1=G4c[:, :, :, 1, :], op=mybir.AluOpType.add)
                    nc.vector.tensor_reduce(
                        out=res[:].rearrange("p f r -> p r f"),
                        in_=G4c[:, :, :, 0, :].rearrange("p c r f -> p r f c"),
                        axis=mybir.AxisListType.X, op=mybir.AluOpType.add)
                else:
                    gv3 = gc[:].rearrange("p (rc v) -> p rc v", v=32)
                    nc.vector.tensor_tensor(
                        out=gv3, in0=gv3,
                        in1=maskt[:].rearrange("p v -> p () v").to_broadcast([P, R * 8, 32]),
                        op=mybir.AluOpType.mult)
                    G = mpool.tile([P, R * 8, 2], f32, name="G")
                    nc.vector.tensor_reduce(
                        out=G[:],
                        in_=gc[:].rearrange("p (rc i f) -> p rc f i", i=16, f=2),
                        axis=mybir.AxisListType.X, op=mybir.AluOpType.add)
                    nc.vector.tensor_tensor(
                        out=G[:], in0=G[:],
                        in1=w8c[:].rearrange("p c r -> p (c r) ()").to_broadcast([P, R * 8, 2]),
                        op=mybir.AluOpType.mult)
                    nc.vector.tensor_reduce(
                        out=res[:].rearrange("p f r -> p r f"),
                        in_=G[:].rearrange("p (c r) f -> p r f c", c=8),
                        axis=mybir.AxisListType.X, op=mybir.AluOpType.add)
                resb = pool.tile([P, 2, R], bf16, name="resb")
                nc.vector.tensor_copy(out=resb[:], in_=res[:])
                nc.scalar.dma_start(
                    out=ptsF[tc_, :, 2 * lc_ * R:(2 * lc_ + 2) * R].rearrange("p (f r) -> p f r", r=R),
                    in_=resb[:])

            pend = []
            for l in range(n_levels):
                n_l = NS[l]
                dense = (n_l + 1) ** 3 <= TBL
                n_elems = (n_l + 1) ** 3 if dense else TBL

                quad = (l == 0)
                tblt = cpool.tile([P, 4913 * 8], bf16, name="tbl", tag="tbl")
                if quad:
                    src = embB4
                    ncols = n_elems * 8
                elif dense:
                    src = embB2[PAIR_LEVELS.index(l)]
                    ncols = n_elems * 4
                else:
                    src = embB[l]
                    ncols = TBL * 2
                if not skip_gather:
                    # split the 8MB broadcast across both DMA-capable
                    # queues (SP + ACT) so it isn't one serialized ring
                    for qi, eng in enumerate([nc.sync, nc.scalar]):
                        eng.dma_start(
                            out=tblt[64 * qi:64 * (qi + 1), 0:ncols],
                            in_=src[None, 0:ncols].to_broadcast([64, ncols]))
                pmt = cpool.tile([P, 6], f32, name=f"pm{l}", tag="pm")
                nc.sync.dma_start(out=pmt[:], in_=pm_d[l, :, :])

                for t in range(nt):
                    if skip_prep:
                        continue
                    xt = mpool.tile([P, R, 6], f32, name="xt")
                    nc.scalar.dma_start(out=xt[:], in_=xv[t])
                    xtT = mpool.tile([P, 6, R], f32, name="xtT")
                    nc.vector.tensor_copy(out=xtT[:], in_=xt[:].rearrange("p r c -> p c r"))
                    if l == 0:
                        viewb = pool.tile([P, 3, R], bf16, name="viewb")
                        nc.vector.tensor_copy(out=viewb[:], in_=xtT[:, 3:6, :])
                        nc.sync.dma_start(
                            out=ptsF[t, :, 32 * R:35 * R].rearrange("p (c r) -> p c r", r=R),
                            in_=viewb[:])
                    # scaled+clamped coords -> xl = clamp((x+5)*0.1) * n_l
                    xl = pool.tile([P, 3, R], f32, name="xl")
                    nc.vector.tensor_scalar(out=xl[:], in0=xtT[:, 0:3, :],
                                            scalar1=5.0, scalar2=0.1,
                                            op0=mybir.AluOpType.add,
                                            op1=mybir.AluOpType.mult)
                    nc.vector.tensor_scalar(out=xl[:], in0=xl[:],
                                            scalar1=0.0, scalar2=CLAMP_HI,
                                            op0=mybir.AluOpType.max,
                                            op1=mybir.AluOpType.min)
                    nc.vector.tensor_scalar_mul(xl[:], xl[:], float(n_l))
                    vi = pool.tile([P, 3, R], i32, name="vi")
                    nc.vector.tensor_copy(out=vi[:], in_=xl[:])
                    vf = pool.tile([P, 3, R], f32, name="vf")
                    nc.vector.tensor_copy(out=vf[:], in_=vi[:])
                    gtt = pool.tile([P, 3, R], f32, name="gtt")
                    nc.vector.tensor_tensor(out=gtt[:], in0=vf[:], in1=xl[:],
                                            op=mybir.AluOpType.is_gt)
                    nc.vector.tensor_tensor(out=vf[:], in0=vf[:], in1=gtt[:],
                                            op=mybir.AluOpType.subtract)
                    wfr = pool.tile([P, 3, R], f32, name="wfr")
                    nc.vector.tensor_tensor(out=wfr[:], in0=xl[:], in1=vf[:],
                                            op=mybir.AluOpType.subtract)
                    # vv6 rows: (v0, v0+1, v1, v1+1, v2, v2+1)
                    vv6 = mpool.tile([P, 6, R], f32, name="vv6")
                    vv6v = vv6[:].rearrange("p (a b) r -> p a b r", b=2)
                    nc.vector.tensor_copy(out=vv6v[:, :, 0, :], in_=vf[:])
                    nc.vector.tensor_scalar_add(vv6v[:, :, 1, :], vf[:], 1.0)
                    m6f = mpool.tile([P, 6, R], f32, name="m6f")
                    nc.vector.tensor_tensor(out=m6f[:], in0=vv6[:],
                                            in1=pmt[:].rearrange("p c -> p c ()").to_broadcast([P, 6, R]),
                                            op=mybir.AluOpType.mult)
                    idx16 = pool.tile([P, 8, R], i16, name="idx16")
                    if dense:
                        c01f = pool.tile([P, 4, R], f32, name="c01f")
                        nc.vector.tensor_tensor(
                            out=c01f[:].rearrange("p (a b) r -> p a b r", b=2),
                            in0=m6f[:, 0:2, :].rearrange("p a r -> p a () r").to_broadcast([P, 2, 2, R]),
                            in1=m6f[:, 2:4, :].rearrange("p a r -> p () a r").to_broadcast([P, 2, 2, R]),
                            op=mybir.AluOpType.add)
                        idx8f = pool.tile([P, 8, R], f32, name="idx8f")
                        nc.vector.tensor_tensor(
                            out=idx8f[:].rearrange("p (a b) r -> p a b r", b=2),
                            in0=c01f[:].rearrange("p a r -> p a () r").to_broadcast([P, 4, 2, R]),
                            in1=m6f[:, 4:6, :].rearrange("p a r -> p () a r").to_broadcast([P, 4, 2, R]),
                            op=mybir.AluOpType.add)
                        if quad:
                            # y0z0 corner per x-corner: positions 0, 4
                            idx16q = pool.tile([P, 2, R], i16, name="idx16q")
                            nc.vector.tensor_copy(
                                out=idx16q[:],
                                in_=idx8f[:].rearrange("p (a u) r -> p a u r", u=4)[:, :, 0, :])
                        else:
                            # even-z corners only: pair table fetches z, z+1
                            idx16p = pool.tile([P, 4, R], i16, name="idx16p")
                            nc.vector.tensor_copy(
                                out=idx16p[:],
                                in_=idx8f[:].rearrange("p (a b) r -> p a b r", b=2)[:, :, 0, :])
                    else:
                        m6i = mpool.tile([P, 6, R], i32, name="m6i")
                        nc.vector.tensor_copy(out=m6i[:], in_=m6f[:])
                        c01 = pool.tile([P, 4, R], i32, name="c01")
                        nc.vector.tensor_tensor(
                            out=c01[:].rearrange("p (a b) r -> p a b r", b=2),
                            in0=m6i[:, 0:2, :].rearrange("p a r -> p a () r").to_broadcast([P, 2, 2, R]),
                            in1=m6i[:, 2:4, :].rearrange("p a r -> p () a r").to_broadcast([P, 2, 2, R]),
                            op=mybir.AluOpType.bitwise_xor)
                        idx8 = mpool.tile([P, 8, R], i32, name="idx8")
                        nc.vector.tensor_tensor(
                            out=idx8[:].rearrange("p (a b) r -> p a b r", b=2),
                            in0=c01[:].rearrange("p a r -> p a () r").to_broadcast([P, 4, 2, R]),
                            in1=m6i[:, 4:6, :].rearrange("p a r -> p () a r").to_broadcast([P, 4, 2, R]),
                            op=mybir.AluOpType.bitwise_xor)
                        nc.vector.tensor_scalar(out=idx8[:], in0=idx8[:],
                                                scalar1=16383, scalar2=None,
                                                op0=mybir.AluOpType.bitwise_and)
                        nc.vector.tensor_copy(out=idx16[:], in_=idx8[:])
                    # trilinear weights w8 rows: (1-w0, w0, 1-w1, w1, 1-w2, w2)
                    ww6 = mpool.tile([P, 6, R], f32, name="ww6")
                    ww6v = ww6[:].rearrange("p (a b) r -> p a b r", b=2)
                    nc.vector.tensor_scalar(out=ww6v[:, :, 0, :], in0=wfr[:],
                                            scalar1=-1.0, scalar2=1.0,
                                            op0=mybir.AluOpType.mult,
                                            op1=mybir.AluOpType.add)
                    nc.vector.tensor_copy(out=ww6v[:, :, 1, :], in_=wfr[:])
                    w01 = pool.tile([P, 4, R], f32, name="w01")
                    nc.vector.tensor_tensor(
                        out=w01[:].rearrange("p (a b) r -> p a b r", b=2),
                        in0=ww6[:, 0:2, :].rearrange("p a r -> p a () r").to_broadcast([P, 2, 2, R]),
                        in1=ww6[:, 2:4, :].rearrange("p a r -> p () a r").to_broadcast([P, 2, 2, R]),
                        op=mybir.AluOpType.mult)
                    w8f = mpool.tile([P, 8, R], f32, name="w8f")
                    nc.vector.tensor_tensor(
                        out=w8f[:].rearrange("p (a b) r -> p a b r", b=2),
                        in0=w01[:].rearrange("p a r -> p a () r").to_broadcast([P, 4, 2, R]),
                        in1=ww6[:, 4:6, :].rearrange("p a r -> p () a r").to_broadcast([P, 4, 2, R]),
                        op=mybir.AluOpType.mult)
                    # ---- gather (Pool) into a static buffer ----
                    gt = gbuf[(l * nt + t) % 2]
                    if not skip_gather:
                        if quad:
                            nc.gpsimd.ap_gather(
                                gt[:, 0:NIDX * 2].rearrange("p (n d) -> p n d", d=8),
                                tblt[:, 0:n_elems * 8].rearrange("p (n d) -> p n d", d=8),
                                idx16q[:].rearrange("p c r -> p (c r)"),
                                channels=P, num_elems=n_elems, d=8, num_idxs=NIDX // 4)
                        elif dense:
                            nc.gpsimd.ap_gather(
                                gt[:, 0:NIDX * 2].rearrange("p (n d) -> p n d", d=4),
                                tblt[:, 0:n_elems * 4].rearrange("p (n d) -> p n d", d=4),
                                idx16p[:].rearrange("p c r -> p (c r)"),
                                channels=P, num_elems=n_elems, d=4, num_idxs=NIDX // 2)
                        else:
                            nc.gpsimd.ap_gather(
                                gt[:, 0:NIDX * 2].rearrange("p (n d) -> p n d", d=2),
                                tblt[:, 0:TBL * 2].rearrange("p (n d) -> p n d", d=2),
                                idx16[:].rearrange("p c r -> p (c r)"),
                                channels=P, num_elems=TBL, d=2, num_idxs=NIDX)
                    # software pipeline: consume the PREVIOUS iteration's
                    # gather so the in-order DVE queue never blocks the prep
                    if quad:
                        w8q = mpool.tile([P, 2, R, 4], f32, name="w8q")
                        nc.vector.tensor_copy(
                            out=w8q[:],
                            in_=w8f[:].rearrange("p (c u) r -> p c r u", u=4))
                        pend.append((gt, w8q, t, l, "quad"))
                    else:
                        pend.append((gt, w8f, t, l, dense))
                    if len(pend) < 2:
                        continue
                    consume(*pend.pop(0))

            while pend:
                consume(*pend.pop(0))

        # ---------------- Phase 2: MLP ----------------
        with tc.tile_pool(name="ph2c", bufs=1) as cpool2, \
             tc.tile_pool(name="ph2", bufs=3) as pool2, \
             tc.tile_pool(name="ph2p", bufs=1, space="PSUM") as ppool:
            dW0b = cpool2.tile([32, 64], bf16, name="dW0b")
            nc.gpsimd.dma_start(out=dW0b[:], in_=w_d["dW0"][:, :])
            dW1b = cpool2.tile([64, 16], bf16, name="dW1b")
            nc.gpsimd.dma_start(out=dW1b[:], in_=w_d["dW1"][:, :])
            cW0d = cpool2.tile([1, 64], bf16, name="cW0d")
            nc.gpsimd.dma_start(out=cW0d[:], in_=w_d["cW0"][0:1, :])
            cW0h = cpool2.tile([15, 64], bf16, name="cW0h")
            nc.gpsimd.dma_start(out=cW0h[:], in_=w_d["cW0"][1:16, :])
            cW0v = cpool2.tile([3, 64], bf16, name="cW0v")
            nc.gpsimd.dma_start(out=cW0v[:], in_=w_d["cW0"][16:19, :])
            cW1b = cpool2.tile([64, 64], bf16, name="cW1b")
            nc.gpsimd.dma_start(out=cW1b[:], in_=w_d["cW1"][:, :])
            cW2b = cpool2.tile([64, 3], bf16, name="cW2b")
            nc.gpsimd.dma_start(out=cW2b[:], in_=w_d["cW2"][:, :])
            db0t = cpool2.tile([64, 1], f32, name="db0t")
            nc.sync.dma_start(out=db0t[:], in_=w_d["db0"][:, None])
            db1d = cpool2.tile([1, 1], f32, name="db1d")
            nc.sync.dma_start(out=db1d[:], in_=w_d["db1"][0:1, None])
            db1r = cpool2.tile([15, 1], f32, name="db1r")
            nc.sync.dma_start(out=db1r[:], in_=w_d["db1"][1:16, None])
            cb0t = cpool2.tile([64, 1], f32, name="cb0t")
            nc.sync.dma_start(out=cb0t[:], in_=w_d["cb0"][:, None])
            cb1t = cpool2.tile([64, 1], f32, name="cb1t")
            nc.sync.dma_start(out=cb1t[:], in_=w_d["cb1"][:, None])
            cb2t = cpool2.tile([3, 1], f32, name="cb2t")
            nc.sync.dma_start(out=cb2t[:], in_=w_d["cb2"][:, None])

            Relu = mybir.ActivationFunctionType.Relu
            Sig = mybir.ActivationFunctionType.Sigmoid
            q_per_t = P * R // NM      # 16 MLP tiles per phase-1 tile
            for m in range(nmt):
                t, q = divmod(m, q_per_t)
                p0 = q * (NM // R)     # 8 partitions per MLP tile
                ptsT = pool2.tile([32, NM], bf16, name="ptsT")
                nc.sync.dma_start(
                    out=ptsT[:].rearrange("a (j r) -> a j r", r=R),
                    in_=ptsF[t, p0:p0 + NM // R, 0:32 * R]
                        .rearrange("j (a r) -> a j r", r=R))
                view3 = pool2.tile([3, NM], bf16, name="view3")
                nc.sync.dma_start(
                    out=view3[:].rearrange("a (j r) -> a j r", r=R),
                    in_=ptsF[t, p0:p0 + NM // R, 32 * R:35 * R]
                        .rearrange("j (a r) -> a j r", r=R))
                h1p = ppool.tile([64, NM], f32, name="h1p")
                nc.tensor.matmul(h1p[:], dW0b[:], ptsT[:], start=True, stop=True)
                h1 = pool2.tile([64, NM], bf16, name="h1")
                nc.scalar.activation(h1[:], h1p[:], Relu, bias=db0t[:])
                h2pd = ppool.tile([1, NM], f32, name="h2pd")
                nc.tensor.matmul(h2pd[:], dW1b[:, 0:1], h1[:], start=True, stop=True)
                h2pr = ppool.tile([15, NM], f32, name="h2pr")
                nc.tensor.matmul(h2pr[:], dW1b[:, 1:16], h1[:], start=True, stop=True)
                den = pool2.tile([1, NM], bf16, name="den")
                nc.scalar.activation(den[:], h2pd[:], Sig, bias=db1d[:])
                hr = pool2.tile([15, NM], bf16, name="hr")
                nc.scalar.activation(hr[:], h2pr[:], Relu, bias=db1r[:])
                c1p = ppool.tile([64, NM], f32, name="c1p")
                nc.tensor.matmul(c1p[:], cW0d[:], den[:], start=True, stop=False)
                nc.tensor.matmul(c1p[:], cW0h[:], hr[:], start=False, stop=False)
                nc.tensor.matmul(c1p[:], cW0v[:], view3[:], start=False, stop=True)
                c1 = pool2.tile([64, NM], bf16, name="c1")
                nc.scalar.activation(c1[:], c1p[:], Relu, bias=cb0t[:])
                c2p = ppool.tile([64, NM], f32, name="c2p")
                nc.tensor.matmul(c2p[:], cW1b[:], c1[:], start=True, stop=True)
                c2 = pool2.tile([64, NM], bf16, name="c2")
                nc.scalar.activation(c2[:], c2p[:], Relu, bias=cb1t[:])
                c3p = ppool.tile([3, NM], f32, name="c3p")
                nc.tensor.matmul(c3p[:], cW2b[:], c2[:], start=True, stop=True)
                outc = pool2.tile([3, NM], f32, name="outc")
                nc.scalar.activation(outc[:], c3p[:], Sig, bias=cb2t[:])
                denf = pool2.tile([1, NM], f32, name="denf")
                nc.vector.tensor_copy(out=denf[:], in_=den[:])
                nc.sync.dma_start(out=out_d[0:1, m * NM:(m + 1) * NM], in_=denf[:])
                nc.sync.dma_start(out=out_d[1:4, m * NM:(m + 1) * NM], in_=outc[:])

    nc.compile()
    return nc


_CACHE = {}


def kernel(**inputs):
    x = np.asarray(inputs["x"], np.float32)
    b = x.shape[0]
    b_nc = b // NCORES
    if b_nc not in _CACHE:
        _CACHE[b_nc] = build(b_nc)
    nc = _CACHE[b_nc]
    from concourse.bass_utils import run_bass_kernel_spmd
    names = ["embed", "dW0", "db0", "dW1", "db1", "cW0", "cb0", "cW1", "cb1",
             "cW2", "cb2"]
    shared = {k: np.ascontiguousarray(np.asarray(inputs[k], np.float32)) for k in names}
    in_maps = []
    for i in range(NCORES):
        m = dict(shared)
        m["x"] = np.ascontiguousarray(x[i * b_nc:(i + 1) * b_nc])
        in_maps.append(m)
    res = run_bass_kernel_spmd(nc, in_maps, core_ids=list(range(NCORES)))
    out = np.concatenate([r["out"].T for r in res.results], axis=0)
    return np.ascontiguousarray(out.astype(np.float32))


if __name__ == "__main__":
    print("built", build(B_NC))
